# revision 1
# baseline (speedup 1.0000x reference)
"""nn_GCNConv Trainium2 Bass kernel (8 NeuronCores, SPMD, no collectives).

Computation: out = segment_sum(features[src], dst, N) @ W + b
  features [10000,128] f32, edge_index [2,640000] i64, W [128,256], b [256]

Sharding strategy (dst-node sharding -> no cross-core reduce needed):
  - 10240 node slots = 80 windows of 128 nodes; core c owns windows
    10c..10c+9 (nodes [1280c, 1280c+1280)).
  - The host groups edges by destination window (this is the edge shard),
    pads each window's edge list to a uniform number of 128-edge chunks
    (pad: src index 0 with local-dst sentinel -1 -> contributes zero).
  - Per core on device, per window:
      * dma_gather     G[e,:]  = feat_bf16[src[e],:]      (SWDGE row gather)
      * DVE is_equal   H[e,j]  = (local_dst[e] == j)      (one-hot, bf16)
      * PE             aggT   += G_chunk^T @ H_chunk       (PSUM f32 accum)
      * PE             out     = aggT^T @ W ; DVE adds b   (f32)
  - Host concatenates the 8 per-core [1280,256] outputs and truncates to
    10000 rows. Only slicing/packing happens on host; all arithmetic on
    feature values runs on device.
"""

import sys

import numpy as np

_TRN_REPO = "/opt/trn_rl_repo"
if _TRN_REPO not in sys.path:
    sys.path.insert(0, _TRN_REPO)

import concourse.bass as bass  # noqa: E402
import concourse.mybir as mybir  # noqa: E402
import concourse.tile as tile  # noqa: E402
from concourse import bacc, bass_utils  # noqa: E402

# ---------------------------------------------------------------------------
# Workaround: this walrus build rejects >1 sync-wait on a CTRL instruction
# ("Too many sync wait commands"). Tile's tail drain attaches a wait for every
# live sem lane to one InstDrain; chunk them across single-wait nops instead.
import re as _re  # noqa: E402

import bass_rust as _bass_rust  # noqa: E402


def _clock_ticks(vc):
    m = _re.search(r"\[([0-9, ]*)\]", repr(vc))
    return [int(x) for x in m.group(1).split(",")] if m.group(1).strip() else []


def _drain_and_barrier(self, tick_clock, wait_clock):
    ticks = _clock_ticks(tick_clock.global_clock)
    nz = [(i, t) for i, t in enumerate(ticks) if t > 0]
    for i, t in nz:
        vc = _bass_rust.VectorClock()
        vc.require_at_least(i, t)
        nop = self.nc.sync.nop(nofuse=True, hint="tail_wait")
        wait_clock.add_sem_waits(nop.ins, tile.ScopedClock({None: vc}))
    self.nc.sync.drain()  # waits already carried by the nops (SP FIFO order)
    self.nc.all_engine_barrier()
    assert self.sems is not None
    popped = self.nc._tile_sem_poison_stack.pop()
    assert popped is self._sem_poison
    self.nc.clear_and_free_semaphores(list(self.sems.allocated().values()))
    self.nc.all_engine_barrier()


tile.TileContext._drain_and_barrier = _drain_and_barrier
# ---------------------------------------------------------------------------

P = 128            # SBUF partitions = window node count = edge chunk size
C_IN = 128
C_OUT = 256
N_NODES = 10000
N_CORES = 8
WPC = 10           # windows per core
MODE = "bf16_pre"  # "f32" | "bf16_pre" | "bf16_dve"
GATHER_GROUP = 8   # 128-idx chunks per dma_gather call (SWDGE ring limit)


def _build_kernel(n_feat_rows: int, wpc: int, nch: int, mode: str):
    idxcols = nch * P // 16
    nc = bacc.Bacc("TRN2", num_swdge_queues=4, dynamic_dma_scratch_size=65536)
    dt = mybir.dt
    mm_dt = dt.float32 if mode == "f32" else dt.bfloat16

    feat = nc.dram_tensor("feat", [n_feat_rows, C_IN], dt.float32, kind="ExternalInput")
    w_d = nc.dram_tensor("w", [C_IN, C_OUT], dt.float32, kind="ExternalInput")
    bb_d = nc.dram_tensor("bb", [P, C_OUT], dt.float32, kind="ExternalInput")
    iota_d = nc.dram_tensor("iota", [P, P], dt.float32, kind="ExternalInput")
    idxs_d = nc.dram_tensor("idxs", [P, wpc * idxcols], dt.int16, kind="ExternalInput")
    dstloc_d = nc.dram_tensor("dstloc", [P, wpc * nch], dt.float32, kind="ExternalInput")
    out_d = nc.dram_tensor("out", [wpc * P, C_OUT], dt.float32, kind="ExternalOutput")
    if mode == "bf16_pre":
        feat_bf = nc.dram_tensor("feat_bf", [n_feat_rows, C_IN], dt.bfloat16)

    with tile.TileContext(nc) as tc:
        with (
            tc.tile_pool(name="consts", bufs=1) as cpool,
            tc.tile_pool(name="g", bufs=3) as gpool,
            tc.tile_pool(name="h", bufs=3) as hpool,
            tc.tile_pool(name="aggs", bufs=2) as apool,
            tc.tile_pool(name="outs", bufs=2) as opool,
            tc.tile_pool(name="psa", bufs=2, space="PSUM") as psa,
            tc.tile_pool(name="pso", bufs=2, space="PSUM") as pso,
        ):
            if mode == "bf16_pre":
                # one-time cast of the gather source via SBUF bounce
                # (DRAM->DRAM SWDGE cast crashes the device on this runtime)
                nb = n_feat_rows // P
                rem = n_feat_rows - nb * P
                CCH = 26
                with tc.tile_pool(name="cast", bufs=2) as castpool:
                    fview = feat[: nb * P].rearrange("(p a) c -> p a c", p=P)
                    bview = feat_bf[: nb * P].rearrange("(p a) c -> p a c", p=P)
                    for a in range(0, nb, CCH):
                        e = min(a + CCH, nb)
                        cf = castpool.tile([P, CCH, C_IN], dt.float32, tag="cf")
                        cb = castpool.tile([P, CCH, C_IN], dt.bfloat16, tag="cb")
                        nc.sync.dma_start(out=cf[:, : e - a, :], in_=fview[:, a:e, :])
                        nc.vector.tensor_copy(cb[:, : e - a, :], cf[:, : e - a, :])
                        nc.sync.dma_start(out=bview[:, a:e, :], in_=cb[:, : e - a, :])
                    if rem:
                        tf = castpool.tile([P, C_IN], dt.float32, tag="tf")
                        tb = castpool.tile([P, C_IN], dt.bfloat16, tag="tb")
                        nc.sync.dma_start(out=tf[:rem], in_=feat[nb * P :])
                        nc.vector.tensor_copy(tb[:rem], tf[:rem])
                        nc.sync.dma_start(out=feat_bf[nb * P :], in_=tb[:rem])

            iota_s = cpool.tile([P, P], dt.float32)
            w_s = cpool.tile([P, C_OUT], dt.float32)
            bb_s = cpool.tile([P, C_OUT], dt.float32)
            idx_s = cpool.tile([P, wpc, idxcols], dt.int16)
            dst_s = cpool.tile([P, wpc, nch], dt.float32)
            nc.sync.dma_start(out=iota_s[:], in_=iota_d[:])
            nc.sync.dma_start(out=w_s[:], in_=w_d[:])
            nc.sync.dma_start(out=bb_s[:], in_=bb_d[:])
            nc.sync.dma_start(out=idx_s[:].rearrange("p w c -> p (w c)"), in_=idxs_d[:])
            nc.sync.dma_start(out=dst_s[:].rearrange("p w c -> p (w c)"), in_=dstloc_d[:])

            for w in range(wpc):
                groups = [
                    (a, min(a + GATHER_GROUP, nch)) for a in range(0, nch, GATHER_GROUP)
                ]
                if mode == "bf16_pre":
                    g_s = gpool.tile([P, nch, P], dt.bfloat16)
                    gsrc, gdst = feat_bf, g_s
                else:
                    g32 = gpool.tile([P, nch, P], dt.float32, tag="g32")
                    gsrc, gdst = feat, g32
                for gi, (a, e) in enumerate(groups):
                    n = (e - a) * P
                    nc.gpsimd.dma_gather(
                        out_ap=gdst[:, a:e, :],
                        in_ap=gsrc[:],
                        idxs_ap=idx_s[:, w, a * 8 : e * 8],
                        num_idxs=n, num_idxs_reg=n, elem_size=C_IN,
                        queue_num=(w * len(groups) + gi) % 4,
                    )
                if mode == "bf16_dve":
                    g_s = gpool.tile([P, nch, P], dt.bfloat16, tag="g16")
                    nc.vector.tensor_copy(g_s[:], g32[:])
                elif mode == "f32":
                    g_s = g32

                h_s = hpool.tile([P, nch, P], mm_dt)
                nc.vector.tensor_tensor(
                    out=h_s[:],
                    in0=iota_s[:, None, :].to_broadcast([P, nch, P]),
                    in1=dst_s[:, w, :, None].to_broadcast([P, nch, P]),
                    op=mybir.AluOpType.is_equal,
                )

                aggt_p = psa.tile([P, P], dt.float32)
                for k in range(nch):
                    nc.tensor.matmul(
                        aggt_p[:],
                        lhsT=g_s[:, k, :],
                        rhs=h_s[:, k, :],
                        start=(k == 0),
                        stop=(k == nch - 1),
                    )

                aggt_s = apool.tile([P, P], dt.float32)
                nc.scalar.copy(aggt_s[:], aggt_p[:])

                out_p = pso.tile([P, C_OUT], dt.float32)
                nc.tensor.matmul(out_p[:], lhsT=aggt_s[:], rhs=w_s[:], start=True, stop=True)

                out_t = opool.tile([P, C_OUT], dt.float32)
                nc.vector.tensor_add(out_t[:], out_p[:], bb_s[:])
                nc.sync.dma_start(out=out_d[w * P : (w + 1) * P, :], in_=out_t[:])

    nc.compile()
    return nc


def _prep_inputs(features, edge_index, W, b, n_cores: int, wpc: int):
    """Host-side sharding: group edges by dst window, pad, build per-core maps."""
    nw_total = n_cores * wpc

    src = np.asarray(edge_index[0], dtype=np.int64)
    dst = np.asarray(edge_index[1], dtype=np.int64)
    win = dst // P
    order = np.argsort(win, kind="stable")
    src_s = src[order].astype(np.int16)
    dl_s = (dst[order] % P).astype(np.float32)
    counts = np.bincount(win, minlength=nw_total)
    offs = np.zeros(nw_total + 1, dtype=np.int64)
    np.cumsum(counts, out=offs[1:])

    nch = max(1, int(np.ceil(counts.max() / P)))
    epw = nch * P
    idx_pad = np.zeros((nw_total, epw), dtype=np.int16)
    dl_pad = np.full((nw_total, epw), -1.0, dtype=np.float32)
    for w in range(nw_total):
        cnt = counts[w]
        idx_pad[w, :cnt] = src_s[offs[w] : offs[w + 1]]
        dl_pad[w, :cnt] = dl_s[offs[w] : offs[w + 1]]

    # idxs: value i at [i%16, i//16] -> [16, epw//16] block, replicated to all
    # 8 GPSIMD-core partition groups (each Q7 core reads its own group)
    idxs_all = np.tile(
        idx_pad.reshape(nw_total, epw // 16, 16).transpose(0, 2, 1), (1, 8, 1)
    )
    # dstloc: value i at [i%128, i//128] -> [128, nch]
    dl_all = dl_pad.reshape(nw_total, nch, P).transpose(0, 2, 1)

    feat_np = np.ascontiguousarray(np.asarray(features, dtype=np.float32))
    w_np = np.ascontiguousarray(np.asarray(W, dtype=np.float32))
    bb_np = np.tile(np.asarray(b, dtype=np.float32)[None, :], (P, 1))
    iota_np = np.tile(np.arange(P, dtype=np.float32)[None, :], (P, 1))

    in_maps = []
    for c in range(n_cores):
        sl = slice(c * wpc, (c + 1) * wpc)
        in_maps.append(
            {
                "feat": feat_np,
                "w": w_np,
                "bb": bb_np,
                "iota": iota_np,
                "idxs": np.ascontiguousarray(
                    idxs_all[sl].transpose(1, 0, 2).reshape(P, -1)
                ),
                "dstloc": np.ascontiguousarray(
                    dl_all[sl].transpose(1, 0, 2).reshape(P, -1)
                ),
            }
        )
    return in_maps, nch


_KERNEL_CACHE: dict = {}


def _get_kernel(nch: int):
    key = (N_NODES, WPC, nch, MODE)
    if key not in _KERNEL_CACHE:
        _KERNEL_CACHE[key] = _build_kernel(N_NODES, WPC, nch, MODE)
    return _KERNEL_CACHE[key]


def kernel(features, edge_index, W, b):
    features = np.asarray(features, dtype=np.float32)
    edge_index = np.asarray(edge_index)
    W = np.asarray(W, dtype=np.float32)
    b = np.asarray(b, dtype=np.float32)
    assert features.shape == (N_NODES, C_IN), features.shape
    assert W.shape == (C_IN, C_OUT) and b.shape == (C_OUT,)

    in_maps, nch = _prep_inputs(features, edge_index, W, b, N_CORES, WPC)
    nc = _get_kernel(nch)
    res = bass_utils.run_bass_kernel_spmd(nc, in_maps, core_ids=list(range(N_CORES)))
    out = np.concatenate([res.results[c]["out"] for c in range(N_CORES)], axis=0)
    return np.ascontiguousarray(out[:N_NODES]).astype(np.float32)



# revision 2
# speedup vs baseline: 3.4383x; 3.4383x over previous
"""nn_GCNConv Trainium2 Bass kernel (8 NeuronCores, SPMD, no collectives).

Computation: out = segment_sum(features[src], dst, N) @ W + b
  features [10000,128] f32, edge_index [2,640000] i64, W [128,256], b [256]

Strategy (dense-adjacency SpMM, dst-node sharding -> no cross-core reduce):
  segment_sum(features[src], dst) == A^T @ features, where A[s,d] is the
  number of edges s->d (an integer count, here always small).  Instead of
  gathering 80k random 256B feature rows per core (SWDGE descriptor-rate
  bound, ~300us), the host builds the dense count matrix A once from
  edge_index (integer work only) and each core STREAMS its dst-slab of A
  sequentially at full HBM bandwidth:

  - dst axis padded to 10240 = 8 cores x 1280 columns; core c owns
    dst [1280c, 1280c+1280).
  - src axis padded to 10112 = 79 chunks of 128 rows.
  - A slab per core: [128 part, 79 chunk, 1280 dst] in fp8e4 (e4m3
    represents small integer counts exactly -> no quantization error).
  - Features are replicated to every core as [128 part, 79 chunk, 128 feat]
    f32 and cast to bf16 on device.
  - PE: aggT[feat,dst] += featbf[:,k,:]^T @ A[:,k,cg] accumulated over the
    79 chunks in PSUM f32 (three 512/512/256-wide column groups, one PSUM
    bank each).
  - out = aggT^T @ W (bf16 matmul) + b (DVE add), DMA'd out per 128-dst
    window.  Host concatenates the 8 per-core [1280,256] outputs and
    truncates to 10000 rows.  Only integer counting/packing happens on the
    host; all float arithmetic on feature values runs on device.
"""

import sys

import numpy as np

_TRN_REPO = "/opt/trn_rl_repo"
if _TRN_REPO not in sys.path:
    sys.path.insert(0, _TRN_REPO)

import ml_dtypes  # noqa: E402

import concourse.bass as bass  # noqa: E402
import concourse.mybir as mybir  # noqa: E402
import concourse.tile as tile  # noqa: E402
from concourse import bacc, bass_utils  # noqa: E402

# ---------------------------------------------------------------------------
# Workaround: this walrus build rejects >1 sync-wait on a CTRL instruction
# ("Too many sync wait commands"). Tile's tail drain attaches a wait for every
# live sem lane to one InstDrain; chunk them across single-wait nops instead.
import re as _re  # noqa: E402

import bass_rust as _bass_rust  # noqa: E402


def _clock_ticks(vc):
    m = _re.search(r"\[([0-9, ]*)\]", repr(vc))
    return [int(x) for x in m.group(1).split(",")] if m.group(1).strip() else []


def _drain_and_barrier(self, tick_clock, wait_clock):
    ticks = _clock_ticks(tick_clock.global_clock)
    nz = [(i, t) for i, t in enumerate(ticks) if t > 0]
    for i, t in nz:
        vc = _bass_rust.VectorClock()
        vc.require_at_least(i, t)
        nop = self.nc.sync.nop(nofuse=True, hint="tail_wait")
        wait_clock.add_sem_waits(nop.ins, tile.ScopedClock({None: vc}))
    self.nc.sync.drain()  # waits already carried by the nops (SP FIFO order)
    self.nc.all_engine_barrier()
    assert self.sems is not None
    popped = self.nc._tile_sem_poison_stack.pop()
    assert popped is self._sem_poison
    self.nc.clear_and_free_semaphores(list(self.sems.allocated().values()))
    self.nc.all_engine_barrier()


tile.TileContext._drain_and_barrier = _drain_and_barrier
# ---------------------------------------------------------------------------

P = 128
C_IN = 128
C_OUT = 256
N_NODES = 10000
N_CORES = 8
WPC = 10                      # dst windows (of 128 nodes) per core
NCH = 79                      # src chunks of 128 (10112 >= 10000)
N_SRC_PAD = NCH * P           # 10112
DST_PC = WPC * P              # 1280 dst columns per core
COL_GROUPS = [(0, 512), (512, 1024), (1024, 1280)]  # one PSUM bank each
ACH = 8                       # A chunks per DMA (10.2KB/partition lines)
FG = 10                      # feat chunks per load+cast group


def _build_kernel():
    nc = bacc.Bacc("TRN2")
    dt = mybir.dt

    featf_d = nc.dram_tensor("featf", [P, NCH, C_IN], dt.float32, kind="ExternalInput")
    a_d = nc.dram_tensor("a", [P, NCH, DST_PC], dt.float8e4, kind="ExternalInput")
    w_d = nc.dram_tensor("w", [C_IN, C_OUT], dt.float32, kind="ExternalInput")
    bb_d = nc.dram_tensor("bb", [P, C_OUT], dt.float32, kind="ExternalInput")
    out_d = nc.dram_tensor("out", [DST_PC, C_OUT], dt.float32, kind="ExternalOutput")

    with tile.TileContext(nc) as tc:
        with (
            tc.tile_pool(name="consts", bufs=1) as cpool,
            tc.tile_pool(name="feat", bufs=1) as fpool,
            tc.tile_pool(name="astream", bufs=3) as apool,
            tc.tile_pool(name="aggs", bufs=1) as spool,
            tc.tile_pool(name="outs", bufs=3) as opool,
            tc.tile_pool(name="psagg", bufs=1, space="PSUM") as psa,
            tc.tile_pool(name="psout", bufs=2, space="PSUM") as pso,
        ):
            w32 = cpool.tile([C_IN, C_OUT], dt.float32)
            wbf = cpool.tile([C_IN, C_OUT], dt.bfloat16)
            bb_s = cpool.tile([P, C_OUT], dt.float32)
            nc.sync.dma_start(out=w32[:], in_=w_d[:])
            nc.sync.dma_start(out=bb_s[:], in_=bb_d[:])
            nc.vector.tensor_copy(wbf[:], w32[:])

            # features: load f32, cast to bf16 on device (groups so the
            # first matmuls don't wait on the whole 5MB load)
            f32t = fpool.tile([P, NCH, C_IN], dt.float32)
            fbf = fpool.tile([P, NCH, C_IN], dt.bfloat16)
            for a0 in range(0, NCH, FG):
                a1 = min(a0 + FG, NCH)
                nc.sync.dma_start(out=f32t[:, a0:a1, :], in_=featf_d[:, a0:a1, :])
                nc.vector.tensor_copy(fbf[:, a0:a1, :], f32t[:, a0:a1, :])

            # stream A and accumulate aggT[feat,dst] over src chunks
            agg_p = psa.tile([P, DST_PC], dt.float32)  # 3 PSUM banks
            for a0 in range(0, NCH, ACH):
                a1 = min(a0 + ACH, NCH)
                at = apool.tile([P, ACH, DST_PC], dt.float8e4, tag="a")
                nc.sync.dma_start(out=at[:, : a1 - a0, :], in_=a_d[:, a0:a1, :])
                for k in range(a0, a1):
                    for c0, c1 in COL_GROUPS:
                        nc.tensor.matmul(
                            agg_p[:, c0:c1],
                            lhsT=fbf[:, k, :],
                            rhs=at[:, k - a0, c0:c1],
                            start=(k == 0),
                            stop=(k == NCH - 1),
                        )

            aggs = spool.tile([P, DST_PC], dt.bfloat16)
            nc.scalar.copy(aggs[:], agg_p[:])

            for w in range(WPC):
                out_p = pso.tile([P, C_OUT], dt.float32)
                nc.tensor.matmul(
                    out_p[:],
                    lhsT=aggs[:, w * P : (w + 1) * P],
                    rhs=wbf[:],
                    start=True,
                    stop=True,
                )
                out_t = opool.tile([P, C_OUT], dt.float32)
                nc.vector.tensor_add(out_t[:], out_p[:], bb_s[:])
                nc.sync.dma_start(out=out_d[w * P : (w + 1) * P, :], in_=out_t[:])

    nc.compile()
    return nc


def _prep_inputs(features, edge_index, W, b):
    """Host-side packing: dense count matrix A from edge_index (integer
    work only), per-core dst slabs, replicated features/weights."""
    src = np.asarray(edge_index[0]).astype(np.int64)
    dst = np.asarray(edge_index[1]).astype(np.int64)

    dst_pad = N_CORES * DST_PC  # 10240
    counts = np.zeros(N_SRC_PAD * dst_pad, dtype=np.uint8)
    np.add.at(counts, src * dst_pad + dst, 1)
    assert counts.max() <= 15  # e4m3 is exact for small ints
    # uint8 -> fp8e4 bytes via lookup (fast, exact)
    lut = np.arange(256, dtype=np.uint8).astype(ml_dtypes.float8_e4m3).view(np.uint8)
    # [src, dst] -> [part, chunk, dst]
    a_view = counts.reshape(NCH, P, dst_pad).transpose(1, 0, 2)

    feat_np = np.zeros((N_SRC_PAD, C_IN), dtype=np.float32)
    feat_np[:N_NODES] = np.asarray(features, dtype=np.float32)
    feat_np = np.ascontiguousarray(feat_np.reshape(NCH, P, C_IN).transpose(1, 0, 2))

    w_np = np.ascontiguousarray(np.asarray(W, dtype=np.float32))
    bb_np = np.tile(np.asarray(b, dtype=np.float32)[None, :], (P, 1))

    in_maps = []
    for c in range(N_CORES):
        a_c = np.ascontiguousarray(a_view[:, :, c * DST_PC : (c + 1) * DST_PC])
        in_maps.append(
            {
                "featf": feat_np,
                "a": lut[a_c].view(ml_dtypes.float8_e4m3),
                "w": w_np,
                "bb": bb_np,
            }
        )
    return in_maps


_KERNEL_CACHE: dict = {}


def _get_kernel():
    if "nc" not in _KERNEL_CACHE:
        _KERNEL_CACHE["nc"] = _build_kernel()
    return _KERNEL_CACHE["nc"]


def kernel(features, edge_index, W, b):
    features = np.asarray(features, dtype=np.float32)
    edge_index = np.asarray(edge_index)
    W = np.asarray(W, dtype=np.float32)
    b = np.asarray(b, dtype=np.float32)
    assert features.shape == (N_NODES, C_IN), features.shape
    assert W.shape == (C_IN, C_OUT) and b.shape == (C_OUT,)

    in_maps = _prep_inputs(features, edge_index, W, b)
    nc = _get_kernel()
    res = bass_utils.run_bass_kernel_spmd(nc, in_maps, core_ids=list(range(N_CORES)))
    out = np.concatenate([res.results[c]["out"] for c in range(N_CORES)], axis=0)
    return np.ascontiguousarray(out[:N_NODES]).astype(np.float32)


# revision 3
# speedup vs baseline: 3.9026x; 1.1350x over previous
"""nn_GCNConv Trainium2 Bass kernel (8 NeuronCores, SPMD, no collectives).

Computation: out = segment_sum(features[src], dst, N) @ W + b
  features [10000,128] f32, edge_index [2,640000] i64, W [128,256], b [256]

Strategy (dense-adjacency SpMM, dst-node sharding -> no cross-core reduce):
  segment_sum(features[src], dst) == A^T @ features, where A[s,d] is the
  number of edges s->d (small integer counts).  Instead of gathering 80k
  random 256B feature rows per core (SWDGE descriptor-rate bound, ~300us),
  the host builds the dense count matrix A once from edge_index (integer
  work only) and each core STREAMS its dst-slab of A sequentially at full
  HBM bandwidth:

  - dst axis padded to 10240 = 8 cores x 1280 columns; core c owns
    dst [1280c, 1280c+1280).
  - src axis padded to 10112 = 79 chunks of 128 rows.
  - A slab per core, split into three column groups (512/512/256 dst) so
    each group's output projection overlaps the next group's accumulation.
    fp8e4 (e4m3) represents the small integer counts exactly -> no
    quantization error on A.
  - Features are replicated to every core as [128 part, 79 chunk, 128 feat]
    f32, cast to bf16 on device (load interleaved with the first A pass).
  - PE per group: aggT[feat,dstg] += featbf[:,k,:]^T @ A_g[:,k,:]
    accumulated over the 79 chunks into one PSUM bank (f32).
  - Projection per 128-dst window (pipelined PSUM->SBUF copies):
    out = aggT^T @ W (bf16) + b (DVE add), DMA'd out per window.
  - Host concatenates the 8 per-core [1280,256] outputs, truncates to
    10000 rows.  Only integer counting/packing happens on the host; all
    float arithmetic on feature values runs on device.
"""

import sys

import numpy as np

_TRN_REPO = "/opt/trn_rl_repo"
if _TRN_REPO not in sys.path:
    sys.path.insert(0, _TRN_REPO)

import ml_dtypes  # noqa: E402

import concourse.bass as bass  # noqa: E402
import concourse.mybir as mybir  # noqa: E402
import concourse.tile as tile  # noqa: E402
from concourse import bacc, bass_utils  # noqa: E402

# ---------------------------------------------------------------------------
# Workaround: this walrus build rejects >1 sync-wait on a CTRL instruction
# ("Too many sync wait commands"). Tile's tail drain attaches a wait for every
# live sem lane to one InstDrain; chunk them across single-wait nops instead.
import re as _re  # noqa: E402

import bass_rust as _bass_rust  # noqa: E402


def _clock_ticks(vc):
    m = _re.search(r"\[([0-9, ]*)\]", repr(vc))
    return [int(x) for x in m.group(1).split(",")] if m.group(1).strip() else []


def _drain_and_barrier(self, tick_clock, wait_clock):
    ticks = _clock_ticks(tick_clock.global_clock)
    nz = [(i, t) for i, t in enumerate(ticks) if t > 0]
    for i, t in nz:
        vc = _bass_rust.VectorClock()
        vc.require_at_least(i, t)
        nop = self.nc.sync.nop(nofuse=True, hint="tail_wait")
        wait_clock.add_sem_waits(nop.ins, tile.ScopedClock({None: vc}))
    self.nc.sync.drain()  # waits already carried by the nops (SP FIFO order)
    self.nc.all_engine_barrier()
    assert self.sems is not None
    popped = self.nc._tile_sem_poison_stack.pop()
    assert popped is self._sem_poison
    self.nc.clear_and_free_semaphores(list(self.sems.allocated().values()))
    self.nc.all_engine_barrier()


tile.TileContext._drain_and_barrier = _drain_and_barrier
# ---------------------------------------------------------------------------

P = 128
C_IN = 128
C_OUT = 256
N_NODES = 10000
N_CORES = 8
WPC = 10                      # dst windows (of 128 nodes) per core
NCH = 79                      # src chunks of 128 (10112 >= 10000)
N_SRC_PAD = NCH * P           # 10112
DST_PC = WPC * P              # 1280 dst columns per core
CG_W = [512, 512, 256]        # column-group widths (<= 1 PSUM bank each)
CG_O = [0, 512, 1024]         # column-group dst offsets
ACH = 16                      # A chunks per DMA
FG = 16                       # feat chunks per load+cast group


def _build_kernel():
    nc = bacc.Bacc("TRN2")
    dt = mybir.dt

    featf_d = nc.dram_tensor("featf", [P, NCH, C_IN], dt.float32, kind="ExternalInput")
    ag_d = [
        nc.dram_tensor(f"a{g}", [P, NCH, CG_W[g]], dt.float8e4, kind="ExternalInput")
        for g in range(3)
    ]
    w_d = nc.dram_tensor("w", [C_IN, C_OUT], dt.float32, kind="ExternalInput")
    bb_d = nc.dram_tensor("bb", [P, C_OUT], dt.float32, kind="ExternalInput")
    out_d = nc.dram_tensor("out", [DST_PC, C_OUT], dt.float32, kind="ExternalOutput")

    with tile.TileContext(nc) as tc:
        with (
            tc.tile_pool(name="consts", bufs=1) as cpool,
            tc.tile_pool(name="feat", bufs=1) as fpool,
            tc.tile_pool(name="astream", bufs=3) as apool,
            tc.tile_pool(name="aggs", bufs=2) as spool,
            tc.tile_pool(name="outs", bufs=3) as opool,
            tc.tile_pool(name="psagg", bufs=2, space="PSUM") as psa,
            tc.tile_pool(name="psout", bufs=2, space="PSUM") as pso,
        ):
            w32 = cpool.tile([C_IN, C_OUT], dt.float32)
            wbf = cpool.tile([C_IN, C_OUT], dt.bfloat16)
            bb_s = cpool.tile([P, C_OUT], dt.float32)

            f32t = fpool.tile([P, NCH, C_IN], dt.float32)
            fbf = fpool.tile([P, NCH, C_IN], dt.bfloat16)
            fgroups = [(a, min(a + FG, NCH)) for a in range(0, NCH, FG)]
            agroups = [(a, min(a + ACH, NCH)) for a in range(0, NCH, ACH)]

            def load_feat_group(i):
                a0, a1 = fgroups[i]
                nc.sync.dma_start(out=f32t[:, a0:a1, :], in_=featf_d[:, a0:a1, :])
                nc.vector.tensor_copy(fbf[:, a0:a1, :], f32t[:, a0:a1, :])

            # head: first feat group + small consts, then interleave the
            # remaining feat loads with the first column group's A stream
            load_feat_group(0)
            nc.sync.dma_start(out=w32[:], in_=w_d[:])
            nc.sync.dma_start(out=bb_s[:], in_=bb_d[:])
            nc.vector.tensor_copy(wbf[:], w32[:])

            for g in range(3):
                wg = CG_W[g]
                agg_p = psa.tile([P, 512], dt.float32, tag="agg")  # 1 bank
                for gi, (a0, a1) in enumerate(agroups):
                    if g == 0 and gi + 1 < len(fgroups):
                        load_feat_group(gi + 1)
                    at = apool.tile([P, ACH, 512], dt.float8e4, tag="a")
                    nc.sync.dma_start(
                        out=at[:, : a1 - a0, :wg], in_=ag_d[g][:, a0:a1, :]
                    )
                    for k in range(a0, a1):
                        nc.tensor.matmul(
                            agg_p[:, :wg],
                            lhsT=fbf[:, k, :],
                            rhs=at[:, k - a0, :wg],
                            start=(k == 0),
                            stop=(k == NCH - 1),
                        )
                # project this group's windows (pipelined 128-col copies)
                for wi in range(wg // P):
                    w = CG_O[g] // P + wi
                    aggs = spool.tile([P, P], dt.bfloat16, tag="aggs", bufs=4)
                    nc.scalar.copy(aggs[:], agg_p[:, wi * P : (wi + 1) * P])
                    out_p = pso.tile([P, C_OUT], dt.float32, tag="op")
                    nc.tensor.matmul(
                        out_p[:], lhsT=aggs[:], rhs=wbf[:], start=True, stop=True
                    )
                    out_t = opool.tile([P, C_OUT], dt.float32, tag="ot")
                    nc.vector.tensor_add(out_t[:], out_p[:], bb_s[:])
                    nc.sync.dma_start(out=out_d[w * P : (w + 1) * P, :], in_=out_t[:])

    nc.compile()
    return nc


def _prep_inputs(features, edge_index, W, b):
    """Host-side packing: dense count matrix A from edge_index (integer
    work only), per-core dst slabs split into column groups, replicated
    features/weights."""
    src = np.asarray(edge_index[0]).astype(np.int64)
    dst = np.asarray(edge_index[1]).astype(np.int64)

    dst_pad = N_CORES * DST_PC  # 10240
    counts = np.zeros(N_SRC_PAD * dst_pad, dtype=np.uint8)
    np.add.at(counts, src * dst_pad + dst, 1)
    assert counts.max() <= 15  # e4m3 is exact for small ints
    # uint8 -> fp8e4 bytes via lookup (fast, exact)
    lut = np.arange(256, dtype=np.uint8).astype(ml_dtypes.float8_e4m3).view(np.uint8)
    # [src, dst] -> [part, chunk, dst]
    a_view = counts.reshape(NCH, P, dst_pad).transpose(1, 0, 2)

    feat_np = np.zeros((N_SRC_PAD, C_IN), dtype=np.float32)
    feat_np[:N_NODES] = np.asarray(features, dtype=np.float32)
    feat_np = np.ascontiguousarray(feat_np.reshape(NCH, P, C_IN).transpose(1, 0, 2))

    w_np = np.ascontiguousarray(np.asarray(W, dtype=np.float32))
    bb_np = np.tile(np.asarray(b, dtype=np.float32)[None, :], (P, 1))

    in_maps = []
    for c in range(N_CORES):
        m = {"featf": feat_np, "w": w_np, "bb": bb_np}
        base = c * DST_PC
        for g in range(3):
            a_c = np.ascontiguousarray(
                a_view[:, :, base + CG_O[g] : base + CG_O[g] + CG_W[g]]
            )
            m[f"a{g}"] = lut[a_c].view(ml_dtypes.float8_e4m3)
        in_maps.append(m)
    return in_maps


_KERNEL_CACHE: dict = {}


def _get_kernel():
    if "nc" not in _KERNEL_CACHE:
        _KERNEL_CACHE["nc"] = _build_kernel()
    return _KERNEL_CACHE["nc"]


def kernel(features, edge_index, W, b):
    features = np.asarray(features, dtype=np.float32)
    edge_index = np.asarray(edge_index)
    W = np.asarray(W, dtype=np.float32)
    b = np.asarray(b, dtype=np.float32)
    assert features.shape == (N_NODES, C_IN), features.shape
    assert W.shape == (C_IN, C_OUT) and b.shape == (C_OUT,)

    in_maps = _prep_inputs(features, edge_index, W, b)
    nc = _get_kernel()
    res = bass_utils.run_bass_kernel_spmd(nc, in_maps, core_ids=list(range(N_CORES)))
    out = np.concatenate([res.results[c]["out"] for c in range(N_CORES)], axis=0)
    return np.ascontiguousarray(out[:N_NODES]).astype(np.float32)
